# revision 3
# baseline (speedup 1.0000x reference)
"""BiLSTM (T=128, B=256, IN=H=512, L=3) Trainium2 Bass kernel, 8-core SPMD.

Strategy:
  - Batch sharded 8 ways (32 seqs/core); each core runs both directions of
    all 3 layers. No collectives.
  - Forward batch occupies partitions 0-31, backward batch partitions 32-63,
    so one instruction covers both directions of a step.
  - Per layer: phase A computes input projections (xproj) with batched
    matmuls (M-tiles of 128 rows = 4 timesteps x 32 batch) into DRAM;
    phase B runs the T-step recurrence, injecting xproj into the PSUM
    accumulation with an identity matmul and streaming Whh^T as the moving
    operand against the stationary transposed hidden state h^T.
  - Ragged sequences: lengths are sorted descending. Forward rows past their
    length keep computing harmless garbage (h is bounded by +-1); backward
    rows are kept exactly zero until they activate by a per-partition mask
    column fused into the sigma(i)*tanh(g) product. Padded output positions
    are zeroed on the host.
  - Matmuls run as float32r (full-rate fp32 storage); set MM_F32R=False to
    fall back to exact (4x slower) fp32.
"""

import os

import numpy as np

import concourse.bacc as bacc
import concourse.bass as bass
import concourse.mybir as mybir
import concourse.tile as tile
from concourse.bass_utils import run_bass_kernel_spmd

F32 = mybir.dt.float32
F32R = mybir.dt.float32r
AF = mybir.ActivationFunctionType
ALU = mybir.AluOpType

T_FULL = 128
B_FULL = 256
IN0 = 512
H = 512
L = 3
NCORES = 8
BS = B_FULL // NCORES  # 32
G4 = 4 * H  # 2048
NCHUNKS = G4 // 512  # 4 PSUM-bank sized gate chunks
HC = H // 128  # 4 h^T partition chunks

MM_F32R = os.environ.get("MM_F32R", "1") == "1"
USE_TILE_POSITION = os.environ.get("USE_TILE_POSITION", "1") == "1"

# dev override: smaller T for fast compile during bring-up (must divide 4)
T = int(os.environ.get("KERNEL_T", str(T_FULL)))

last_results = None  # BassKernelResults of the most recent run (for test.py)


MDT = F32R if MM_F32R else F32


def _gate_perm():
    """Row permutation taking pytorch gate order (i,f,g,o) to (i,f,o,g)."""
    idx = np.arange(G4)
    return np.concatenate([idx[0:H], idx[H : 2 * H], idx[3 * H : 4 * H], idx[2 * H : 3 * H]])


def build_program(n_steps: int):
    """Emit the full 3-layer BiLSTM program for one core. Returns nc."""
    nc = bacc.Bacc()
    Tn = n_steps
    MT = Tn * BS // 128  # M-tiles per layer-dir in phase A

    # ---- I/O ----
    xT = nc.dram_tensor("xT", [IN0, Tn * BS], MDT, kind="ExternalInput")
    maskd = nc.dram_tensor("maskd", [64, Tn], F32, kind="ExternalInput")
    wih = {}
    whh = {}
    for l in range(L):
        ind = IN0 if l == 0 else 2 * H
        for d in ("f", "b"):
            wih[(l, d)] = nc.dram_tensor(f"wih{l}{d}", [ind, G4], MDT, kind="ExternalInput")
            whh[(l, d)] = nc.dram_tensor(f"whh{l}{d}", [H, G4], MDT, kind="ExternalInput")
    out = nc.dram_tensor("out", [Tn * BS, 2 * H], F32, kind="ExternalOutput")

    ident_d = nc.inline_tensor(np.eye(64, dtype=np.float32), name="ident")

    with tile.TileContext(nc) as tc:
        with (
            tc.tile_pool(name="const", bufs=1) as constp,
            tc.tile_pool(name="dram", bufs=2, space="DRAM") as dramp,
        ):
            i64 = constp.tile([64, 64], F32, name="i64")
            nc.sync.dma_start(i64[:], ident_d[:])
            i64r = constp.tile([64, 64], MDT, name="i64r")
            nc.vector.tensor_copy(i64r[:], i64[:])
            msk = constp.tile([64, Tn], F32, name="msk")
            nc.sync.dma_start(msk[:], maskd[:])

            xt_prev_f = None
            xt_prev_b = None
            for l in range(L):
                ind = IN0 if l == 0 else 2 * H
                KC = ind // 128

                xp_f = dramp.tile([Tn * BS, G4], MDT, name="xp_f")
                xp_b = dramp.tile([Tn * BS, G4], MDT, name="xp_b")
                if l < L - 1:
                    xt_nf = dramp.tile([H, Tn * BS], MDT, name="xt_nf")
                    xt_nb = dramp.tile([H, Tn * BS], MDT, name="xt_nb")
                else:
                    xt_nf = xt_nb = None

                def lhs_src(c):
                    # k-chunk c of the transposed layer input
                    if l == 0:
                        return xT[c * 128 : (c + 1) * 128, :]
                    if c < HC:
                        return xt_prev_f[c * 128 : (c + 1) * 128, :]
                    cc = c - HC
                    return xt_prev_b[cc * 128 : (cc + 1) * 128, :]

                # ================= PHASE A: xproj =================
                with (
                    tc.tile_pool(name="wpoolA", bufs=2) as wpoolA,
                    tc.tile_pool(name="lhsp", bufs=2 * KC + 2) as lhsp,
                    tc.tile_pool(name="psA", bufs=2, space="PSUM") as psA,
                    tc.tile_pool(name="outA", bufs=3) as outA,
                ):
                    wsb = {}
                    for d in ("f", "b"):
                        w = wpoolA.tile([128, KC * G4], MDT, name="wihsb")
                        for c in range(KC):
                            nc.sync.dma_start(
                                w[:, c * G4 : (c + 1) * G4],
                                wih[(l, d)][c * 128 : (c + 1) * 128, :],
                            )
                        wsb[d] = w
                    for m in range(MT):
                        lts = []
                        for c in range(KC):
                            lt = lhsp.tile([128, 128], MDT, name="lhst")
                            nc.sync.dma_start(lt[:], lhs_src(c)[:, m * 128 : (m + 1) * 128])
                            lts.append(lt)
                        for d, xp in (("f", xp_f), ("b", xp_b)):
                            ps = psA.tile([128, G4], F32, name="psa", space="PSUM")
                            for c in range(KC):
                                for n in range(NCHUNKS):
                                    ns = slice(n * 512, (n + 1) * 512)
                                    nc.tensor.matmul(
                                        ps[:, ns],
                                        (lts[c][:]),
                                        (wsb[d][:, c * G4 + n * 512 : c * G4 + (n + 1) * 512]),
                                        start=(c == 0),
                                        stop=(c == KC - 1),
                                    )
                            so = outA.tile([128, G4], MDT, name="soA")
                            nc.vector.tensor_copy(so[:], ps[:])
                            nc.sync.dma_start(xp[m * 128 : (m + 1) * 128, :], so[:])

                # ================= PHASE B: recurrence =================
                with (
                    tc.tile_pool(name="wpoolB", bufs=2) as wpoolB,
                    tc.tile_pool(name="xpsp", bufs=4) as xpsp,
                    tc.tile_pool(name="gps", bufs=1, space="PSUM") as gps,
                    tc.tile_pool(name="tps", bufs=2, space="PSUM") as tps,
                    tc.tile_pool(name="cellp", bufs=2) as cellp,
                    tc.tile_pool(name="statep", bufs=1) as statep,
                ):
                    wsbB = {}
                    for d in ("f", "b"):
                        w = wpoolB.tile([128, HC * G4], MDT, name="whhsb")
                        for c in range(HC):
                            nc.sync.dma_start(
                                w[:, c * G4 : (c + 1) * G4],
                                whh[(l, d)][c * 128 : (c + 1) * 128, :],
                            )
                        wsbB[d] = w
                    hTf = statep.tile([128, HC * 64], MDT, name="hTf")
                    hTb = statep.tile([128, HC * 64], MDT, name="hTb")
                    cst = statep.tile([64, H], F32, name="cst")
                    hb = statep.tile([64, H], F32, name="hb")
                    nc.vector.memset(hTf[:].bitcast(F32), 0.0)
                    nc.vector.memset(hTb[:].bitcast(F32), 0.0)
                    nc.vector.memset(cst[:], 0.0)
                    nc.vector.memset(hb[:], 0.0)

                    for s in range(Tn):
                        tf = s
                        tb = Tn - 1 - s
                        xps = xpsp.tile([64, G4], MDT, name="xps")
                        nc.sync.dma_start(xps[0:32, :], xp_f[tf * BS : (tf + 1) * BS, :])
                        nc.sync.dma_start(xps[32:64, :], xp_b[tb * BS : (tb + 1) * BS, :])

                        G = gps.tile([64, G4], F32, name="G", space="PSUM")
                        for n in range(NCHUNKS):
                            ns = slice(n * 512, (n + 1) * 512)
                            nc.tensor.matmul(
                                G[:, ns], i64r[:], xps[:, ns], start=True, stop=False
                            )
                            for c in range(HC):
                                rh = slice(c * G4 + n * 512, c * G4 + (n + 1) * 512)
                                cs = slice(c * 64, (c + 1) * 64)
                                nc.tensor.matmul(
                                    G[:, ns], hTf[:, cs], wsbB["f"][:, rh],
                                    start=False, stop=False,
                                )
                                nc.tensor.matmul(
                                    G[:, ns], hTb[:, cs], wsbB["b"][:, rh],
                                    start=False, stop=(c == HC - 1),
                                )

                        # ---- cell math (gate order i,f,o,g) ----
                        gt = cellp.tile([64, H], F32, name="gt")
                        nc.scalar.activation(gt[:], G[:, 3 * H : 4 * H], AF.Tanh)
                        ifo = cellp.tile([64, 3 * H], F32, name="ifo")
                        nc.scalar.activation(ifo[:], G[:, 0 : 3 * H], AF.Sigmoid)
                        p = cellp.tile([64, H], F32, name="p")
                        nc.vector.scalar_tensor_tensor(
                            p[:], ifo[:, 0:H], msk[:, s : s + 1], gt[:], ALU.mult, ALU.mult
                        )
                        q = cellp.tile([64, H], F32, name="q")
                        nc.vector.tensor_mul(q[:], ifo[:, H : 2 * H], cst[:])
                        nc.vector.tensor_add(cst[:], p[:], q[:])
                        tch = cellp.tile([64, H], F32, name="tch")
                        nc.scalar.activation(tch[:], cst[:], AF.Tanh)
                        nc.vector.tensor_mul(hb[:], ifo[:, 2 * H : 3 * H], tch[:])

                        # ---- h -> h^T via PE transpose ----
                        TP = tps.tile([128, HC * 64], F32, name="TP", space="PSUM")
                        for c in range(HC):
                            nc.tensor.transpose(
                                TP[:, c * 64 : (c + 1) * 64],
                                hb[:, c * 128 : (c + 1) * 128],
                                i64[:],
                            )
                        tp4 = TP[:].rearrange("p (c x) -> p c x", c=HC)
                        hf4 = hTf[:].rearrange("p (c x) -> p c x", c=HC)
                        hb4 = hTb[:].rearrange("p (c x) -> p c x", c=HC)
                        nc.vector.tensor_copy(hf4[:, :, 0:32], tp4[:, :, 0:32])
                        nc.vector.tensor_copy(hb4[:, :, 32:64], tp4[:, :, 32:64])

                        # ---- write step outputs ----
                        if l < L - 1:
                            for c in range(HC):
                                nc.sync.dma_start(
                                    xt_nf[c * 128 : (c + 1) * 128, tf * BS : (tf + 1) * BS],
                                    hTf[:, c * 64 : c * 64 + 32],
                                )
                                nc.sync.dma_start(
                                    xt_nb[c * 128 : (c + 1) * 128, tb * BS : (tb + 1) * BS],
                                    hTb[:, c * 64 + 32 : c * 64 + 64],
                                )
                        else:
                            nc.sync.dma_start(out[tf * BS : (tf + 1) * BS, 0:H], hb[0:32, :])
                            nc.sync.dma_start(
                                out[tb * BS : (tb + 1) * BS, H : 2 * H], hb[32:64, :]
                            )

                xt_prev_f, xt_prev_b = xt_nf, xt_nb

    nc.compile()
    return nc


def _prep_inputs(x, lengths, params, n_steps):
    """Build per-core input maps. x: [T,B,IN] f32, lengths: [B] sorted desc."""
    Tn = n_steps
    x = np.asarray(x, dtype=np.float32)[:Tn]
    lengths = np.minimum(np.asarray(lengths).astype(np.int64), Tn)
    perm = _gate_perm()

    weights = {}
    for l, layer in enumerate(params):
        for d in ("f", "b"):
            p = {k: np.asarray(v, dtype=np.float32) for k, v in layer[d].items()}
            if np.abs(p["b"]).max() != 0:
                raise NotImplementedError("nonzero LSTM bias not supported")
            weights[f"wih{l}{d}"] = np.ascontiguousarray(p["Wih"][perm].T)
            weights[f"whh{l}{d}"] = np.ascontiguousarray(p["Whh"][perm].T)

    in_maps = []
    for k in range(NCORES):
        sl = slice(k * BS, (k + 1) * BS)
        xs = x[:, sl, :]  # [T, 32, IN]
        xTk = np.ascontiguousarray(xs.transpose(2, 0, 1).reshape(IN0, Tn * BS))
        lens = lengths[sl]
        m = np.ones((64, Tn), dtype=np.float32)
        steps = np.arange(Tn)[None, :]
        m[32:64] = (steps >= (Tn - lens[:, None])).astype(np.float32)
        im = {"xT": xTk, "maskd": m}
        im.update(weights)
        in_maps.append(im)
    return in_maps, lengths


def kernel(x, lengths, params):
    global last_results
    n_steps = T
    in_maps, lens = _prep_inputs(x, lengths, params, n_steps)
    nc = build_program(n_steps)
    res = run_bass_kernel_spmd(nc, in_maps, core_ids=list(range(NCORES)), trace=False)
    last_results = res
    outs = [r["out"].reshape(n_steps, BS, 2 * H) for r in res.results]
    full = np.concatenate(outs, axis=1)  # [T, B, 1024]
    mask = (np.arange(n_steps)[:, None] < lens[None, :]).astype(np.float32)
    full = full * mask[:, :, None]
    return full
